# revision 10
# baseline (speedup 1.0000x reference)
"""2-layer GCN (GCNConv -> ReLU -> GCNConv -> ReLU -> Linear) on 8 TRN2 NeuronCores.

Strategy: shard nodes (and their incident in-edges) across the 8 cores.
Per GCN layer each core computes its shard of the dinv-scaled features
h_s = (x @ W) * dinv, the shards are AllGathered into a full feature table
in DRAM, and each core aggregates messages for its destination shard by
(a) dma_gather of h_s[src] rows into SBUF (128 edges per partition-group),
(b) a one-hot scatter-matmul per 128-edge group accumulating into PSUM.
Edge lists are bucketed host-side by (dst tile, src table-half) and padded
so all 8 cores run an identical SPMD program.
"""

import numpy as np

import concourse.bass as bass
import concourse.tile as tile
from concourse import bacc, mybir, bass_utils
from concourse.masks import make_identity

P = 128
PAD_LANE = 1000.0


class Cfg:
    def __init__(self, N=50000, F_IN=256, F=128, CORES=8, LOW_ROWS=32768,
                 CHUNK_GROUPS=32, SINGLE_PACKET=False):
        assert N % CORES == 0
        self.N = N
        self.F_IN = F_IN
        self.F = F
        self.CORES = CORES
        self.NPC = N // CORES
        self.TILES = (self.NPC + P - 1) // P
        self.NPC_PAD = self.TILES * P
        self.TABLE_ROWS = CORES * self.NPC_PAD
        self.LOW_ROWS = min(LOW_ROWS, self.TABLE_ROWS)
        self.HIGH_ROWS = self.TABLE_ROWS - self.LOW_ROWS
        self.CHUNK_GROUPS = CHUNK_GROUPS
        self.SINGLE_PACKET = SINGLE_PACKET


def _wrap_idx_chunk(idx16):
    """int16 idx list (len % 16 == 0) -> [128, n/16] wrapped + 8x replicated."""
    n = idx16.shape[0]
    assert n % 16 == 0
    a = idx16.reshape(n // 16, 16).T  # [16, n/16]
    return np.tile(a, (8, 1)).astype(np.int16)


def _preprocess(cfg, edge_index):
    """Bucket edges by (core, src-half, dst tile); build per-core device arrays.

    Returns (meta, per_core):
      meta: program-structure constants (identical across cores)
      per_core: list of dicts of named np arrays for in_maps
    """
    src = np.asarray(edge_index[0]).astype(np.int64)
    dst = np.asarray(edge_index[1]).astype(np.int64)
    N, CORES, NPC, NPC_PAD, TILES = cfg.N, cfg.CORES, cfg.NPC, cfg.NPC_PAD, cfg.TILES

    deg = 1.0 + np.bincount(dst, minlength=N).astype(np.float32)
    dinv = (1.0 / np.sqrt(deg)).astype(np.float32)

    src_pad = (src // NPC) * NPC_PAD + (src % NPC)       # padded global row
    core_of = dst // NPC
    dst_local = dst % NPC
    tl = dst_local // P
    lane = (dst_local % P).astype(np.float32)
    hi = (src_pad >= cfg.LOW_ROWS).astype(np.int64)

    # group counts per (core, tile, stream); pad to the max across cores
    counts = np.zeros((CORES, TILES, 2), dtype=np.int64)
    np.add.at(counts, (core_of, tl, hi), 1)
    groups = np.ceil(counts / P).astype(np.int64).max(axis=0)  # [TILES, 2]
    groups[:, 0] = np.maximum(groups[:, 0], 1)  # >=1 low group per tile
    GL, GH = groups[:, 0], groups[:, 1]
    GLsum, GHsum = int(GL.sum()), int(GH.sum())
    Lg0 = np.concatenate([[0], np.cumsum(GL)])[:TILES]   # group offset per tile
    Hg0 = np.concatenate([[0], np.cumsum(GH)])[:TILES]

    # sort edges by (core, stream, tile) for contiguous slicing
    order = np.lexsort((tl, hi, core_of))
    s_src, s_hi, s_core, s_tl, s_lane = (
        src_pad[order], hi[order], core_of[order], tl[order], lane[order])

    per_core = []
    seg_starts = {}
    # compute segment boundaries: key (core, hi, tile)
    keys = s_core * (2 * TILES) + s_hi * TILES + s_tl
    uniq, first = np.unique(keys, return_index=True)
    seg_len = np.diff(np.concatenate([first, [len(keys)]]))
    for k, f, ln in zip(uniq, first, seg_len):
        seg_starts[int(k)] = (int(f), int(ln))

    for c in range(CORES):
        idxs = {0: np.zeros(GLsum * P, np.int64), 1: np.zeros(GHsum * P, np.int64)}
        lanes = {0: np.full(GLsum * P, PAD_LANE, np.float32),
                 1: np.full(GHsum * P, PAD_LANE, np.float32)}
        for s, g0s, gcnt in ((0, Lg0, GL), (1, Hg0, GH)):
            for t in range(TILES):
                key = c * (2 * TILES) + s * TILES + t
                if key not in seg_starts:
                    continue
                f, ln = seg_starts[key]
                pos = int(g0s[t]) * P
                vals = s_src[f:f + ln]
                if s == 1:
                    vals = vals - cfg.LOW_ROWS
                idxs[s][pos:pos + ln] = vals
                lanes[s][pos:pos + ln] = s_lane[f:f + ln]

        d = {}
        # wrapped idx arrays, chunked
        for s, name, gsum in ((0, "idxL", GLsum), (1, "idxH", GHsum)):
            chunks = []
            for g0 in range(0, gsum, cfg.CHUNK_GROUPS):
                g1 = min(g0 + cfg.CHUNK_GROUPS, gsum)
                chunks.append(_wrap_idx_chunk(idxs[s][g0 * P:g1 * P].astype(np.int16)))
            d[name] = (np.concatenate(chunks, axis=1) if chunks
                       else np.zeros((P, 0), np.int16))
        d["lanesL"] = np.ascontiguousarray(lanes[0].reshape(GLsum, P).T)
        d["lanesH"] = (np.ascontiguousarray(lanes[1].reshape(GHsum, P).T)
                       if GHsum else np.zeros((P, 0), np.float32))
        dl = np.ones(NPC_PAD, np.float32)
        dl[:NPC] = dinv[c * NPC:(c + 1) * NPC]
        d["dinv"] = np.ascontiguousarray(dl.reshape(TILES, P).T)
        per_core.append(d)

    # chunk tables: (stream, ci, g0, g1); first-need tile for issue ordering
    def tile_of_group(g0s, gcnt, g):
        t = int(np.searchsorted(np.cumsum(gcnt), g, side="right"))
        return t

    chunk_list = []
    chunk_of = [{}, {}]
    for s, gsum, gcnt in ((0, GLsum, GL), (1, GHsum, GH)):
        for ci, g0 in enumerate(range(0, gsum, cfg.CHUNK_GROUPS)):
            g1 = min(g0 + cfg.CHUNK_GROUPS, gsum)
            ft = tile_of_group(None, gcnt, g0)
            chunk_list.append((s, ci, g0, g1, ft))
            for g in range(g0, g1):
                chunk_of[s][g] = (ci, g - g0)
    chunk_list.sort(key=lambda e: (e[4], e[0], e[1]))

    meta = {
        "GL": GL.tolist(), "GH": GH.tolist(),
        "Lg0": Lg0.tolist(), "Hg0": Hg0.tolist(),
        "GLsum": GLsum, "GHsum": GHsum,
        "chunks": chunk_list, "chunk_of": chunk_of,
    }
    return meta, per_core, dinv


def _build_program(cfg, meta, bl_value, _mode=None):
    f32 = mybir.dt.float32
    F, F_IN, TILES, CORES = cfg.F, cfg.F_IN, cfg.TILES, cfg.CORES
    GLsum, GHsum = meta["GLsum"], meta["GHsum"]
    GL, GH = meta["GL"], meta["GH"]
    Lg0, Hg0 = meta["Lg0"], meta["Hg0"]
    KCH = F_IN // P  # K chunks for layer 1

    nc = bacc.Bacc("TRN2", target_bir_lowering=False, debug=False,
                   num_devices=CORES)
    xT_d = nc.dram_tensor("xT", [F_IN, cfg.NPC_PAD], f32, kind="ExternalInput")
    w1_d = nc.dram_tensor("W1", [F_IN, F], f32, kind="ExternalInput")
    w2_d = nc.dram_tensor("W2", [F, F], f32, kind="ExternalInput")
    wl_d = nc.dram_tensor("Wl", [F, 1], f32, kind="ExternalInput")
    b1_d = nc.dram_tensor("b1t", [P, F], f32, kind="ExternalInput")
    b2_d = nc.dram_tensor("b2t", [P, F], f32, kind="ExternalInput")
    dinv_d = nc.dram_tensor("dinv", [P, TILES], f32, kind="ExternalInput")
    iota_d = nc.dram_tensor("iota", [P, 8 * P], f32, kind="ExternalInput")
    idxL_d = nc.dram_tensor("idxL", [P, GLsum * 8], mybir.dt.int16, kind="ExternalInput")
    idxH_d = nc.dram_tensor("idxH", [P, max(GHsum, 1) * 8], mybir.dt.int16, kind="ExternalInput")
    lanesL_d = nc.dram_tensor("lanesL", [P, GLsum], f32, kind="ExternalInput")
    lanesH_d = nc.dram_tensor("lanesH", [P, max(GHsum, 1)], f32, kind="ExternalInput")
    y_d = nc.dram_tensor("y", [P, TILES], f32, kind="ExternalOutput")

    with tile.TileContext(nc) as tc:
        with tc.tile_pool(name="dram", bufs=1, space="DRAM") as dpool, \
             tc.tile_pool(name="const", bufs=1) as cpool, \
             tc.tile_pool(name="hsp", bufs=TILES) as hs_pool, \
             tc.tile_pool(name="rTp", bufs=TILES) as rT_pool, \
             tc.tile_pool(name="lhs", bufs=4) as lhs_pool, \
             tc.tile_pool(name="gatL", bufs=2) as gatL_pool, \
             tc.tile_pool(name="gatH", bufs=2) as gatH_pool, \
             tc.tile_pool(name="Sp", bufs=3) as S_pool, \
             tc.tile_pool(name="ep", bufs=3) as ep_pool, \
             tc.tile_pool(name="pA", bufs=2, space="PSUM") as pA, \
             tc.tile_pool(name="pB", bufs=2, space="PSUM") as pB, \
             tc.tile_pool(name="pT", bufs=2, space="PSUM") as pT, \
             tc.tile_pool(name="pY", bufs=2, space="PSUM") as pY:

            # ---- constants ----
            w1_t = cpool.tile([P, KCH, F], f32)
            for k in range(KCH):
                nc.sync.dma_start(out=w1_t[:, k, :], in_=w1_d[k * P:(k + 1) * P, :])
            w2_t = cpool.tile([P, F], f32)
            nc.sync.dma_start(out=w2_t[:], in_=w2_d[:, :])
            wl_t = cpool.tile([P, 1], f32)
            nc.sync.dma_start(out=wl_t[:], in_=wl_d[:, :])
            b1_t = cpool.tile([P, F], f32)
            nc.sync.dma_start(out=b1_t[:], in_=b1_d[:, :])
            b2_t = cpool.tile([P, F], f32)
            nc.sync.dma_start(out=b2_t[:], in_=b2_d[:, :])
            dinv_t = cpool.tile([P, TILES], f32)
            nc.sync.dma_start(out=dinv_t[:], in_=dinv_d[:, :])
            iota_t = cpool.tile([P, 8 * P], f32)
            nc.sync.dma_start(out=iota_t[:], in_=iota_d[:, :])
            idxL_t = cpool.tile([P, GLsum * 8], mybir.dt.int16)
            nc.sync.dma_start(out=idxL_t[:], in_=idxL_d[:, :])
            idxH_t = cpool.tile([P, max(GHsum, 1) * 8], mybir.dt.int16)
            nc.sync.dma_start(out=idxH_t[:], in_=idxH_d[:, :])
            lanesL_t = cpool.tile([P, GLsum], f32)
            nc.sync.dma_start(out=lanesL_t[:], in_=lanesL_d[:, :])
            lanesH_t = cpool.tile([P, max(GHsum, 1)], f32)
            nc.sync.dma_start(out=lanesH_t[:], in_=lanesH_d[:, :])
            ident = cpool.tile([P, P], f32)
            make_identity(nc, ident[:])

            b_tiles = [b1_t, b2_t]
            rT_tiles = [None] * TILES
            layer_bufs = []

            for l in range(2):
                ag_in = dpool.tile([cfg.NPC_PAD, F], f32, name=f"ag_in{l}")
                ag_out = dpool.tile([cfg.TABLE_ROWS, F], f32,
                                    addr_space="Shared", name=f"ag_out{l}")
                layer_bufs.append((ag_in, ag_out))

                # ---- phase A: h_s = (x @ W) * dinv, write shard table ----
                hs_tiles = []
                for t in range(TILES):
                    hpsum = pA.tile([P, F], f32, tag="hpsum", name=f"hps{l}_{t}")
                    if l == 0:
                        for k in range(KCH):
                            xt = lhs_pool.tile([P, P], f32, tag="xt",
                                               name=f"xt{l}_{t}_{k}")
                            nc.sync.dma_start(
                                out=xt[:],
                                in_=xT_d[k * P:(k + 1) * P, t * P:(t + 1) * P])
                            nc.tensor.matmul(hpsum[:], lhsT=xt[:],
                                             rhs=w1_t[:, k, :],
                                             start=(k == 0), stop=(k == KCH - 1))
                    else:
                        nc.tensor.matmul(hpsum[:], lhsT=rT_tiles[t][:],
                                         rhs=w2_t[:], start=True, stop=True)
                    hs_t = hs_pool.tile([P, F], f32, tag="hs", name=f"hs{l}_{t}")
                    nc.vector.tensor_scalar(out=hs_t[:], in0=hpsum[:],
                                            scalar1=dinv_t[:, t:t + 1],
                                            scalar2=None,
                                            op0=mybir.AluOpType.mult)
                    nc.sync.dma_start(out=ag_in[t * P:(t + 1) * P, :], in_=hs_t[:])
                    hs_tiles.append(hs_t)

                nc.gpsimd.collective_compute(
                    "AllGather", mybir.AluOpType.bypass,
                    replica_groups=[list(range(CORES))],
                    ins=[ag_in[:].opt()], outs=[ag_out[:].opt()])

                if _mode == "ag_only":
                    jt = cpool.tile([P, TILES], f32, name="jt")
                    nc.sync.dma_start(out=jt[:], in_=ag_out[0:P, 0:TILES])
                    nc.sync.dma_start(out=y_d[:, :], in_=jt[:])
                    break

                # ---- phase B: gather + one-hot scatter matmuls ----
                gtiles = [{}, {}]
                for (s, ci, g0, g1, _ft) in meta["chunks"]:
                    ng = g1 - g0
                    pool = gatL_pool if s == 0 else gatH_pool
                    gt = pool.tile([P, ng, F], f32, tag=f"g{s}",
                                   name=f"g{l}_{s}_{ci}",
                                   padded_shape=[P, cfg.CHUNK_GROUPS, F])
                    idx_t = idxL_t if s == 0 else idxH_t
                    view = (ag_out[0:cfg.LOW_ROWS, :] if s == 0
                            else ag_out[cfg.LOW_ROWS:cfg.TABLE_ROWS, :])
                    nc.gpsimd.dma_gather(
                        out_ap=gt[:], in_ap=view,
                        idxs_ap=idx_t[:, g0 * 8:g1 * 8],
                        num_idxs=ng * P, num_idxs_reg=ng * P, elem_size=F,
                        single_packet=cfg.SINGLE_PACKET)
                    gtiles[s][ci] = gt

                if _mode == "nomm":
                    junk = cpool.tile([P, cfg.F], f32, name="junk")
                    for s in (0, 1):
                        for gt in gtiles[s].values():
                            nc.vector.tensor_copy(out=junk[:], in_=gt[:, 0, :])
                    jt2 = cpool.tile([P, TILES], f32, name="jt2")
                    nc.vector.tensor_copy(out=jt2[:], in_=junk[:, 0:TILES])
                    nc.sync.dma_start(out=y_d[:, :], in_=jt2[:])
                    break

                Sblocks = [{}, {}]

                def get_S(s, b, l=l, Sblocks=Sblocks):
                    if b not in Sblocks[s]:
                        lan = lanesL_t if s == 0 else lanesH_t
                        gsum = GLsum if s == 0 else GHsum
                        g0, g1 = b * 8, min(b * 8 + 8, gsum)
                        st = S_pool.tile([P, (g1 - g0) * P], f32, tag="S",
                                         name=f"S{l}_{s}_{b}",
                                         padded_shape=[P, 8 * P])
                        nc.vector.tensor_tensor(
                            out=st[:],
                            in0=lan[:, g0:g1].to_broadcast([P, g1 - g0, P]),
                            in1=iota_t[:, :(g1 - g0) * P],
                            op=mybir.AluOpType.is_equal)
                        Sblocks[s][b] = st
                    return Sblocks[s][b]

                new_rT = [None] * TILES
                for t in range(TILES):
                    apsum = pB.tile([P, F], f32, tag="apsum", name=f"aps{l}_{t}")
                    seq = ([(0, g) for g in range(Lg0[t], Lg0[t] + GL[t])]
                           + [(1, g) for g in range(Hg0[t], Hg0[t] + GH[t])])
                    for k, (s, g) in enumerate(seq):
                        ci, slot = meta["chunk_of"][s][g]
                        st = get_S(s, g // 8)
                        j = g - (g // 8) * 8
                        nc.tensor.matmul(
                            apsum[:], lhsT=st[:, j * P:(j + 1) * P],
                            rhs=gtiles[s][ci][:, slot, :],
                            start=(k == 0), stop=(k == len(seq) - 1))
                    # epilogue: r = relu((psum + hs) * dinv + b)
                    sm = ep_pool.tile([P, F], f32, tag="ep1", name=f"sm{l}_{t}")
                    nc.vector.tensor_tensor(out=sm[:], in0=apsum[:],
                                            in1=hs_tiles[t][:],
                                            op=mybir.AluOpType.add)
                    pr = ep_pool.tile([P, F], f32, tag="ep2", name=f"pr{l}_{t}")
                    nc.vector.tensor_scalar(out=pr[:], in0=sm[:],
                                            scalar1=dinv_t[:, t:t + 1],
                                            scalar2=None,
                                            op0=mybir.AluOpType.mult)
                    bi = ep_pool.tile([P, F], f32, tag="ep3", name=f"bi{l}_{t}")
                    nc.vector.tensor_tensor(out=bi[:], in0=pr[:],
                                            in1=b_tiles[l][:],
                                            op=mybir.AluOpType.add)
                    r = ep_pool.tile([P, F], f32, tag="ep4", name=f"r{l}_{t}")
                    nc.scalar.activation(out=r[:], in_=bi[:],
                                         func=mybir.ActivationFunctionType.Relu)
                    tp = pT.tile([P, P], f32, tag="tp", name=f"tp{l}_{t}")
                    nc.tensor.transpose(out=tp[:], in_=r[:], identity=ident[:])
                    rT_t = rT_pool.tile([P, P], f32, tag="rT", name=f"rT{l}_{t}")
                    nc.vector.tensor_copy(out=rT_t[:], in_=tp[:])
                    new_rT[t] = rT_t
                rT_tiles = new_rT

            # ---- final linear: y = r2 @ Wl + bl ----
            if _mode is not None:
                rT_tiles = []
            y_sb = cpool.tile([P, TILES], f32, name="y_sb")
            for t in range(TILES if _mode is None else 0):
                yp = pY.tile([P, 1], f32, tag="yp", name=f"yp{t}")
                nc.tensor.matmul(yp[:], lhsT=rT_tiles[t][:], rhs=wl_t[:],
                                 start=True, stop=True)
                nc.vector.tensor_scalar(out=y_sb[:, t:t + 1], in0=yp[:],
                                        scalar1=float(bl_value), scalar2=None,
                                        op0=mybir.AluOpType.add)
            if _mode is None:
                nc.sync.dma_start(out=y_d[:, :], in_=y_sb[:])

    nc.compile()
    return nc


def _make_in_maps(cfg, per_core, x, W1, b1, W2, b2, Wl):
    iota = np.tile(np.arange(P, dtype=np.float32), (P, 8))
    b1t = np.tile(np.asarray(b1, np.float32).reshape(1, -1), (P, 1))
    b2t = np.tile(np.asarray(b2, np.float32).reshape(1, -1), (P, 1))
    in_maps = []
    for c in range(cfg.CORES):
        d = per_core[c]
        xT = np.zeros((cfg.F_IN, cfg.NPC_PAD), np.float32)
        xT[:, :cfg.NPC] = np.asarray(x[c * cfg.NPC:(c + 1) * cfg.NPC], np.float32).T
        gh = d["idxH"].shape[1] // 8
        in_maps.append({
            "xT": np.ascontiguousarray(xT),
            "W1": np.asarray(W1, np.float32),
            "W2": np.asarray(W2, np.float32),
            "Wl": np.asarray(Wl, np.float32).reshape(cfg.F, 1),
            "b1t": b1t, "b2t": b2t,
            "dinv": d["dinv"],
            "iota": np.ascontiguousarray(iota),
            "idxL": d["idxL"],
            "idxH": (d["idxH"] if gh else np.zeros((P, 8), np.int16)),
            "lanesL": d["lanesL"],
            "lanesH": (d["lanesH"] if d["lanesH"].shape[1]
                       else np.full((P, 1), PAD_LANE, np.float32)),
        })
    return in_maps


_CACHE = {}


def _get_compiled(cfg, edge_index):
    key = hash(np.asarray(edge_index).tobytes())
    if key not in _CACHE:
        meta, per_core, dinv = _preprocess(cfg, edge_index)
        _CACHE[key] = (meta, per_core, dinv)
    return _CACHE[key]


def kernel(x, edge_index, W1, b1, W2, b2, Wl, bl, _cfg=None, _run=None):
    cfg = _cfg or Cfg()
    x = np.asarray(x, np.float32)
    meta, per_core, _dinv = _get_compiled(cfg, edge_index)
    bl_value = float(np.asarray(bl).reshape(-1)[0])
    nc = _build_program(cfg, meta, bl_value)
    in_maps = _make_in_maps(cfg, per_core, x, W1, b1, W2, b2, Wl)
    if _run is not None:
        results = _run(nc, in_maps)
    else:
        res = bass_utils.run_bass_kernel_spmd(
            nc, in_maps, core_ids=list(range(cfg.CORES)))
        results = res.results
    y = np.zeros((cfg.N, 1), np.float32)
    for c in range(cfg.CORES):
        yc = results[c]["y"]  # [P, TILES]
        y[c * cfg.NPC:(c + 1) * cfg.NPC, 0] = yc.T.reshape(cfg.NPC_PAD)[:cfg.NPC]
    return y


# revision 15
# speedup vs baseline: 1.1422x; 1.1422x over previous
"""2-layer GCN (GCNConv -> ReLU -> GCNConv -> ReLU -> Linear) on 8 TRN2 NeuronCores.

Strategy: shard nodes (and their incident in-edges) across the 8 cores.
Per GCN layer each core computes its shard of the dinv-scaled features
h_s = (x @ W) * dinv, the shards are AllGathered into a full feature table
in DRAM, and each core aggregates messages for its destination shard by
(a) dma_gather of h_s[src] rows into SBUF (128 edges per partition-group),
(b) a one-hot scatter-matmul per 128-edge group accumulating into PSUM.
Edge lists are bucketed host-side by (dst tile, src table-half) and padded
so all 8 cores run an identical SPMD program.
"""

import numpy as np

import concourse.bass as bass
import concourse.tile as tile
from concourse import bacc, mybir, bass_utils
from concourse.masks import make_identity

P = 128
PAD_LANE = 1000.0


class Cfg:
    def __init__(self, N=50000, F_IN=256, F=128, CORES=8, LOW_ROWS=32768,
                 CHUNK_GROUPS=32, SINGLE_PACKET=False):
        assert N % CORES == 0
        self.N = N
        self.F_IN = F_IN
        self.F = F
        self.CORES = CORES
        self.NPC = N // CORES
        self.TILES = (self.NPC + P - 1) // P
        self.NPC_PAD = self.TILES * P
        self.TABLE_ROWS = CORES * self.NPC_PAD
        self.LOW_ROWS = min(LOW_ROWS, self.TABLE_ROWS)
        self.HIGH_ROWS = self.TABLE_ROWS - self.LOW_ROWS
        self.CHUNK_GROUPS = CHUNK_GROUPS
        self.SINGLE_PACKET = SINGLE_PACKET


def _wrap_idx_chunk(idx16):
    """int16 idx list (len % 16 == 0) -> [128, n/16] wrapped + 8x replicated."""
    n = idx16.shape[0]
    assert n % 16 == 0
    a = idx16.reshape(n // 16, 16).T  # [16, n/16]
    return np.tile(a, (8, 1)).astype(np.int16)


def _preprocess(cfg, edge_index):
    """Bucket edges by (core, src-half, dst tile); build per-core device arrays.

    Returns (meta, per_core):
      meta: program-structure constants (identical across cores)
      per_core: list of dicts of named np arrays for in_maps
    """
    src = np.asarray(edge_index[0]).astype(np.int64)
    dst = np.asarray(edge_index[1]).astype(np.int64)
    N, CORES, NPC, NPC_PAD, TILES = cfg.N, cfg.CORES, cfg.NPC, cfg.NPC_PAD, cfg.TILES

    deg = 1.0 + np.bincount(dst, minlength=N).astype(np.float32)
    dinv = (1.0 / np.sqrt(deg)).astype(np.float32)

    # Balance in-degree across the 8 cores per tile index: snake-deal nodes
    # (sorted by in-degree desc) into CORES*TILES buckets of 128 slots. This
    # equalizes per-(core,tile,stream) edge counts so the SPMD max-over-cores
    # group padding shrinks.
    NB = CORES * TILES
    order_nodes = np.argsort(-(deg), kind="stable")
    i = np.arange(N)
    rnd, idx = i // NB, i % NB
    bucket = np.where(rnd % 2 == 0, idx, NB - 1 - idx)
    slot = rnd
    assert slot.max() < P, "bucket overflow"
    c_of = bucket % CORES
    t_of = bucket // CORES
    pos = np.empty(N, np.int64)
    pos[order_nodes] = c_of * NPC_PAD + t_of * P + slot

    src_pad = pos[src]                                   # padded global row
    core_of = pos[dst] // NPC_PAD
    dst_local = pos[dst] % NPC_PAD
    tl = dst_local // P
    lane = (dst_local % P).astype(np.float32)
    hi = (src_pad >= cfg.LOW_ROWS).astype(np.int64)

    # group counts per (core, tile, stream); pad to the max across cores
    counts = np.zeros((CORES, TILES, 2), dtype=np.int64)
    np.add.at(counts, (core_of, tl, hi), 1)
    groups = np.ceil(counts / P).astype(np.int64).max(axis=0)  # [TILES, 2]
    groups[:, 0] = np.maximum(groups[:, 0], 1)  # >=1 low group per tile
    GL, GH = groups[:, 0], groups[:, 1]
    GLsum, GHsum = int(GL.sum()), int(GH.sum())
    Lg0 = np.concatenate([[0], np.cumsum(GL)])[:TILES]   # group offset per tile
    Hg0 = np.concatenate([[0], np.cumsum(GH)])[:TILES]

    # sort edges by (core, stream, tile) for contiguous slicing
    order = np.lexsort((tl, hi, core_of))
    s_src, s_hi, s_core, s_tl, s_lane = (
        src_pad[order], hi[order], core_of[order], tl[order], lane[order])

    per_core = []
    seg_starts = {}
    # compute segment boundaries: key (core, hi, tile)
    keys = s_core * (2 * TILES) + s_hi * TILES + s_tl
    uniq, first = np.unique(keys, return_index=True)
    seg_len = np.diff(np.concatenate([first, [len(keys)]]))
    for k, f, ln in zip(uniq, first, seg_len):
        seg_starts[int(k)] = (int(f), int(ln))

    for c in range(CORES):
        idxs = {0: np.zeros(GLsum * P, np.int64), 1: np.zeros(GHsum * P, np.int64)}
        lanes = {0: np.full(GLsum * P, PAD_LANE, np.float32),
                 1: np.full(GHsum * P, PAD_LANE, np.float32)}
        for s, g0s, gcnt in ((0, Lg0, GL), (1, Hg0, GH)):
            for t in range(TILES):
                key = c * (2 * TILES) + s * TILES + t
                if key not in seg_starts:
                    continue
                f, ln = seg_starts[key]
                wp = int(g0s[t]) * P
                vals = s_src[f:f + ln]
                if s == 1:
                    vals = vals - cfg.LOW_ROWS
                idxs[s][wp:wp + ln] = vals
                lanes[s][wp:wp + ln] = s_lane[f:f + ln]

        d = {}
        # wrapped idx arrays, chunked
        for s, name, gsum in ((0, "idxL", GLsum), (1, "idxH", GHsum)):
            chunks = []
            for g0 in range(0, gsum, cfg.CHUNK_GROUPS):
                g1 = min(g0 + cfg.CHUNK_GROUPS, gsum)
                chunks.append(_wrap_idx_chunk(idxs[s][g0 * P:g1 * P].astype(np.int16)))
            d[name] = (np.concatenate(chunks, axis=1) if chunks
                       else np.zeros((P, 0), np.int16))
        d["lanesL"] = np.ascontiguousarray(lanes[0].reshape(GLsum, P).T)
        d["lanesH"] = (np.ascontiguousarray(lanes[1].reshape(GHsum, P).T)
                       if GHsum else np.zeros((P, 0), np.float32))
        dl = np.ones(NPC_PAD, np.float32)
        mine = pos // NPC_PAD == c
        dl[pos[mine] % NPC_PAD] = dinv[mine]
        d["dinv"] = np.ascontiguousarray(dl.reshape(TILES, P).T)
        d["_pos"] = pos
        per_core.append(d)

    # chunk tables: (stream, ci, g0, g1); first-need tile for issue ordering
    def tile_of_group(g0s, gcnt, g):
        t = int(np.searchsorted(np.cumsum(gcnt), g, side="right"))
        return t

    chunk_list = []
    chunk_of = [{}, {}]
    for s, gsum, gcnt in ((0, GLsum, GL), (1, GHsum, GH)):
        for ci, g0 in enumerate(range(0, gsum, cfg.CHUNK_GROUPS)):
            g1 = min(g0 + cfg.CHUNK_GROUPS, gsum)
            ft = tile_of_group(None, gcnt, g0)
            chunk_list.append((s, ci, g0, g1, ft))
            for g in range(g0, g1):
                chunk_of[s][g] = (ci, g - g0)
    chunk_list.sort(key=lambda e: (e[4], e[0], e[1]))

    meta = {
        "GL": GL.tolist(), "GH": GH.tolist(),
        "Lg0": Lg0.tolist(), "Hg0": Hg0.tolist(),
        "GLsum": GLsum, "GHsum": GHsum,
        "chunks": chunk_list, "chunk_of": chunk_of, "pos": pos,
    }
    return meta, per_core, dinv


def _build_program(cfg, meta, bl_value, _mode=None):
    f32 = mybir.dt.float32
    F, F_IN, TILES, CORES = cfg.F, cfg.F_IN, cfg.TILES, cfg.CORES
    GLsum, GHsum = meta["GLsum"], meta["GHsum"]
    GL, GH = meta["GL"], meta["GH"]
    Lg0, Hg0 = meta["Lg0"], meta["Hg0"]
    KCH = F_IN // P  # K chunks for layer 1

    nc = bacc.Bacc("TRN2", target_bir_lowering=False, debug=False,
                   num_devices=CORES)
    xT_d = nc.dram_tensor("xT", [F_IN, cfg.NPC_PAD], f32, kind="ExternalInput")
    w1_d = nc.dram_tensor("W1", [F_IN, F], f32, kind="ExternalInput")
    w2_d = nc.dram_tensor("W2", [F, F], f32, kind="ExternalInput")
    wl_d = nc.dram_tensor("Wl", [F, 1], f32, kind="ExternalInput")
    b1_d = nc.dram_tensor("b1t", [1, F], f32, kind="ExternalInput")
    b2_d = nc.dram_tensor("b2t", [1, F], f32, kind="ExternalInput")
    rdinv_d = nc.dram_tensor("rdinv", [1, cfg.NPC_PAD], f32, kind="ExternalInput")
    dinv_d = nc.dram_tensor("dinv", [P, TILES], f32, kind="ExternalInput")
    iota_d = nc.dram_tensor("iota", [P, 8 * P], f32, kind="ExternalInput")
    idxL_d = nc.dram_tensor("idxL", [P, GLsum * 8], mybir.dt.int16, kind="ExternalInput")
    idxH_d = nc.dram_tensor("idxH", [P, max(GHsum, 1) * 8], mybir.dt.int16, kind="ExternalInput")
    lanesL_d = nc.dram_tensor("lanesL", [P, GLsum], f32, kind="ExternalInput")
    lanesH_d = nc.dram_tensor("lanesH", [P, max(GHsum, 1)], f32, kind="ExternalInput")
    y_d = nc.dram_tensor("y", [P, TILES], f32, kind="ExternalOutput")

    with tile.TileContext(nc) as tc:
        with tc.tile_pool(name="dram", bufs=1, space="DRAM") as dpool, \
             tc.tile_pool(name="const", bufs=1) as cpool, \
             tc.tile_pool(name="hsp", bufs=TILES) as hs_pool, \
             tc.tile_pool(name="rTp", bufs=TILES) as rT_pool, \
             tc.tile_pool(name="lhs", bufs=4) as lhs_pool, \
             tc.tile_pool(name="gatL", bufs=2) as gatL_pool, \
             tc.tile_pool(name="gatH", bufs=2) as gatH_pool, \
             tc.tile_pool(name="Sp", bufs=3) as S_pool, \
             tc.tile_pool(name="ep", bufs=3) as ep_pool, \
             tc.tile_pool(name="pA", bufs=2, space="PSUM") as pA, \
             tc.tile_pool(name="pB", bufs=2, space="PSUM") as pB, \
             tc.tile_pool(name="pT", bufs=2, space="PSUM") as pT, \
             tc.tile_pool(name="pY", bufs=2, space="PSUM") as pY:

            # ---- constants ----
            w1_t = cpool.tile([P, KCH, F], f32)
            for k in range(KCH):
                nc.sync.dma_start(out=w1_t[:, k, :], in_=w1_d[k * P:(k + 1) * P, :])
            w2_t = cpool.tile([P, F], f32)
            nc.sync.dma_start(out=w2_t[:], in_=w2_d[:, :])
            wl_t = cpool.tile([P, 1], f32)
            nc.sync.dma_start(out=wl_t[:], in_=wl_d[:, :])
            b1_t = cpool.tile([1, F], f32)
            nc.sync.dma_start(out=b1_t[:], in_=b1_d[:, :])
            b2_t = cpool.tile([1, F], f32)
            nc.sync.dma_start(out=b2_t[:], in_=b2_d[:, :])
            rdinv_t = cpool.tile([1, cfg.NPC_PAD], f32)
            nc.sync.dma_start(out=rdinv_t[:], in_=rdinv_d[:, :])
            dinv_t = cpool.tile([P, TILES], f32)
            nc.sync.dma_start(out=dinv_t[:], in_=dinv_d[:, :])
            iota_t = cpool.tile([P, 8 * P], f32)
            nc.sync.dma_start(out=iota_t[:], in_=iota_d[:, :])
            idxL_t = cpool.tile([P, GLsum * 8], mybir.dt.int16)
            nc.sync.dma_start(out=idxL_t[:], in_=idxL_d[:, :])
            idxH_t = cpool.tile([P, max(GHsum, 1) * 8], mybir.dt.int16)
            nc.sync.dma_start(out=idxH_t[:], in_=idxH_d[:, :])
            lanesL_t = cpool.tile([P, GLsum], f32)
            nc.sync.dma_start(out=lanesL_t[:], in_=lanesL_d[:, :])
            lanesH_t = cpool.tile([P, max(GHsum, 1)], f32)
            nc.sync.dma_start(out=lanesH_t[:], in_=lanesH_d[:, :])
            ident = cpool.tile([P, P], f32)
            make_identity(nc, ident[:])
            ident_bf = cpool.tile([P, P], mybir.dt.bfloat16)
            make_identity(nc, ident_bf[:])

            b_tiles = [b1_t, b2_t]
            rT_tiles = [None] * TILES
            layer_bufs = []

            for l in range(2):
                ag_in = dpool.tile([cfg.NPC_PAD, F], mybir.dt.bfloat16,
                                   name=f"ag_in{l}")
                ag_out = dpool.tile([cfg.TABLE_ROWS, F], mybir.dt.bfloat16,
                                    addr_space="Shared", name=f"ag_out{l}")
                layer_bufs.append((ag_in, ag_out))

                # ---- phase A: h_s = (x @ W) * dinv, write shard table ----
                hs_tiles = []
                for t in range(TILES):
                    hpsum = pA.tile([P, F], f32, tag="hpsum", name=f"hps{l}_{t}")
                    if l == 0:
                        for k in range(KCH):
                            xt = lhs_pool.tile([P, P], f32, tag="xt",
                                               name=f"xt{l}_{t}_{k}")
                            nc.sync.dma_start(
                                out=xt[:],
                                in_=xT_d[k * P:(k + 1) * P, t * P:(t + 1) * P])
                            nc.tensor.matmul(hpsum[:], lhsT=xt[:],
                                             rhs=w1_t[:, k, :],
                                             start=(k == 0), stop=(k == KCH - 1))
                    else:
                        nc.tensor.matmul(hpsum[:], lhsT=rT_tiles[t][:],
                                         rhs=w2_t[:], start=True, stop=True)
                    hs_t = hs_pool.tile([P, F], mybir.dt.bfloat16, tag="hs",
                                        name=f"hs{l}_{t}")
                    nc.scalar.activation(
                        out=hs_t[:], in_=hpsum[:],
                        func=mybir.ActivationFunctionType.Copy,
                        scale=dinv_t[:, t:t + 1])
                    nc.sync.dma_start(out=ag_in[t * P:(t + 1) * P, :], in_=hs_t[:])
                    hs_tiles.append(hs_t)

                nc.gpsimd.collective_compute(
                    "AllGather", mybir.AluOpType.bypass,
                    replica_groups=[list(range(CORES))],
                    ins=[ag_in[:].opt()], outs=[ag_out[:].opt()])

                if _mode == "ag_only":
                    jt = cpool.tile([P, TILES], f32, name="jt")
                    nc.sync.dma_start(out=jt[:], in_=ag_out[0:P, 0:TILES])
                    nc.sync.dma_start(out=y_d[:, :], in_=jt[:])
                    break

                # ---- phase B: gather + one-hot scatter matmuls ----
                gtiles = [{}, {}]
                for (s, ci, g0, g1, _ft) in meta["chunks"]:
                    ng = g1 - g0
                    pool = gatL_pool if s == 0 else gatH_pool
                    gt = pool.tile([P, ng, F], mybir.dt.bfloat16, tag=f"g{s}",
                                   name=f"g{l}_{s}_{ci}",
                                   padded_shape=[P, cfg.CHUNK_GROUPS, F])
                    idx_t = idxL_t if s == 0 else idxH_t
                    view = (ag_out[0:cfg.LOW_ROWS, :] if s == 0
                            else ag_out[cfg.LOW_ROWS:cfg.TABLE_ROWS, :])
                    nc.gpsimd.dma_gather(
                        out_ap=gt[:], in_ap=view,
                        idxs_ap=idx_t[:, g0 * 8:g1 * 8],
                        num_idxs=ng * P, num_idxs_reg=ng * P, elem_size=F,
                        single_packet=cfg.SINGLE_PACKET)
                    gtiles[s][ci] = gt

                if _mode == "nomm":
                    junk = cpool.tile([P, cfg.F], f32, name="junk")
                    for s in (0, 1):
                        for gt in gtiles[s].values():
                            nc.vector.tensor_copy(out=junk[:], in_=gt[:, 0, :])
                    jt2 = cpool.tile([P, TILES], f32, name="jt2")
                    nc.vector.tensor_copy(out=jt2[:], in_=junk[:, 0:TILES])
                    nc.sync.dma_start(out=y_d[:, :], in_=jt2[:])
                    break

                Sblocks = [{}, {}]

                def get_S(s, b, l=l, Sblocks=Sblocks):
                    if b not in Sblocks[s]:
                        lan = lanesL_t if s == 0 else lanesH_t
                        gsum = GLsum if s == 0 else GHsum
                        g0, g1 = b * 8, min(b * 8 + 8, gsum)
                        st = S_pool.tile([P, (g1 - g0) * P], mybir.dt.bfloat16,
                                         tag="S", name=f"S{l}_{s}_{b}",
                                         padded_shape=[P, 8 * P])
                        nc.vector.tensor_tensor(
                            out=st[:],
                            in0=lan[:, g0:g1].to_broadcast([P, g1 - g0, P]),
                            in1=iota_t[:, :(g1 - g0) * P],
                            op=mybir.AluOpType.is_equal)
                        Sblocks[s][b] = st
                    return Sblocks[s][b]

                new_rT = [None] * TILES
                for t in range(TILES):
                    apsum = pB.tile([P, F], f32, tag="apsum", name=f"aps{l}_{t}")
                    seq = ([(0, g) for g in range(Lg0[t], Lg0[t] + GL[t])]
                           + [(1, g) for g in range(Hg0[t], Hg0[t] + GH[t])])
                    for k, (s, g) in enumerate(seq):
                        ci, slot = meta["chunk_of"][s][g]
                        st = get_S(s, g // 8)
                        j = g - (g // 8) * 8
                        nc.tensor.matmul(
                            apsum[:], lhsT=st[:, j * P:(j + 1) * P],
                            rhs=gtiles[s][ci][:, slot, :],
                            start=(k == 0), stop=False)
                    # self-loop term: psum += I @ hs  (PE accumulate)
                    nc.tensor.matmul(apsum[:], lhsT=ident_bf[:],
                                     rhs=hs_tiles[t][:], start=False, stop=False)
                    # bias pre-divided by dinv: psum += outer(1/dinv_t, b)
                    nc.tensor.matmul(apsum[:],
                                     lhsT=rdinv_t[0:1, t * P:(t + 1) * P],
                                     rhs=b_tiles[l][:], start=False, stop=True)
                    # r = relu(psum * dinv)
                    r = ep_pool.tile([P, F], f32, tag="ep4", name=f"r{l}_{t}")
                    nc.scalar.activation(out=r[:], in_=apsum[:],
                                         func=mybir.ActivationFunctionType.Relu,
                                         scale=dinv_t[:, t:t + 1])
                    tp = pT.tile([P, P], f32, tag="tp", name=f"tp{l}_{t}")
                    nc.tensor.transpose(out=tp[:], in_=r[:], identity=ident[:])
                    rT_t = rT_pool.tile([P, P], f32, tag="rT", name=f"rT{l}_{t}")
                    nc.vector.tensor_copy(out=rT_t[:], in_=tp[:])
                    new_rT[t] = rT_t
                rT_tiles = new_rT

            # ---- final linear: y = r2 @ Wl + bl ----
            if _mode is not None:
                rT_tiles = []
            y_sb = cpool.tile([P, TILES], f32, name="y_sb")
            for t in range(TILES if _mode is None else 0):
                yp = pY.tile([P, 1], f32, tag="yp", name=f"yp{t}")
                nc.tensor.matmul(yp[:], lhsT=rT_tiles[t][:], rhs=wl_t[:],
                                 start=True, stop=True)
                nc.vector.tensor_scalar(out=y_sb[:, t:t + 1], in0=yp[:],
                                        scalar1=float(bl_value), scalar2=None,
                                        op0=mybir.AluOpType.add)
            if _mode is None:
                nc.sync.dma_start(out=y_d[:, :], in_=y_sb[:])

    nc.compile()
    return nc


def _make_in_maps(cfg, per_core, x, W1, b1, W2, b2, Wl):
    iota = np.tile(np.arange(P, dtype=np.float32), (P, 8))
    b1t = np.asarray(b1, np.float32).reshape(1, -1)
    b2t = np.asarray(b2, np.float32).reshape(1, -1)
    in_maps = []
    for c in range(cfg.CORES):
        d = per_core[c]
        xT = np.zeros((cfg.F_IN, cfg.NPC_PAD), np.float32)
        pos = per_core[c]["_pos"]
        mine = pos // cfg.NPC_PAD == c
        xT[:, pos[mine] % cfg.NPC_PAD] = np.asarray(x, np.float32)[mine].T
        rdinv = (1.0 / d["dinv"]).T.reshape(1, cfg.NPC_PAD).astype(np.float32)
        gh = d["idxH"].shape[1] // 8
        in_maps.append({
            "rdinv": np.ascontiguousarray(rdinv),
            "xT": np.ascontiguousarray(xT),
            "W1": np.asarray(W1, np.float32),
            "W2": np.asarray(W2, np.float32),
            "Wl": np.asarray(Wl, np.float32).reshape(cfg.F, 1),
            "b1t": b1t, "b2t": b2t,
            "dinv": d["dinv"],
            "iota": np.ascontiguousarray(iota),
            "idxL": d["idxL"],
            "idxH": (d["idxH"] if gh else np.zeros((P, 8), np.int16)),
            "lanesL": d["lanesL"],
            "lanesH": (d["lanesH"] if d["lanesH"].shape[1]
                       else np.full((P, 1), PAD_LANE, np.float32)),
        })
    return in_maps


_CACHE = {}


def _get_compiled(cfg, edge_index):
    key = hash(np.asarray(edge_index).tobytes())
    if key not in _CACHE:
        meta, per_core, dinv = _preprocess(cfg, edge_index)
        _CACHE[key] = (meta, per_core, dinv)
    return _CACHE[key]


def kernel(x, edge_index, W1, b1, W2, b2, Wl, bl, _cfg=None, _run=None):
    cfg = _cfg or Cfg()
    x = np.asarray(x, np.float32)
    meta, per_core, _dinv = _get_compiled(cfg, edge_index)
    bl_value = float(np.asarray(bl).reshape(-1)[0])
    nc = _build_program(cfg, meta, bl_value)
    in_maps = _make_in_maps(cfg, per_core, x, W1, b1, W2, b2, Wl)
    if _run is not None:
        results = _run(nc, in_maps)
    else:
        res = bass_utils.run_bass_kernel_spmd(
            nc, in_maps, core_ids=list(range(cfg.CORES)))
        results = res.results
    pos = meta["pos"]
    y_pad = np.zeros(cfg.CORES * cfg.NPC_PAD, np.float32)
    for c in range(cfg.CORES):
        yc = results[c]["y"]  # [P, TILES]
        y_pad[c * cfg.NPC_PAD:(c + 1) * cfg.NPC_PAD] = yc.T.reshape(cfg.NPC_PAD)
    return y_pad[pos].reshape(cfg.N, 1).astype(np.float32)


# revision 16
# speedup vs baseline: 1.3658x; 1.1958x over previous
"""2-layer GCN (GCNConv -> ReLU -> GCNConv -> ReLU -> Linear) on 8 TRN2 NeuronCores.

Strategy: shard nodes (and their incident in-edges) across the 8 cores.
Per GCN layer each core computes its shard of the dinv-scaled features
h_s = (x @ W) * dinv, the shards are AllGathered into a full feature table
in DRAM, and each core aggregates messages for its destination shard by
(a) dma_gather of h_s[src] rows into SBUF (128 edges per partition-group),
(b) a one-hot scatter-matmul per 128-edge group accumulating into PSUM.
Edge lists are bucketed host-side by (dst tile, src table-half) and padded
so all 8 cores run an identical SPMD program.
"""

import numpy as np

import concourse.bass as bass
import concourse.tile as tile
from concourse import bacc, mybir, bass_utils
from concourse.masks import make_identity

P = 128
PAD_LANE = 1000.0


class Cfg:
    def __init__(self, N=50000, F_IN=256, F=128, CORES=8, LOW_ROWS=32768,
                 CHUNK_GROUPS=32, SINGLE_PACKET=False):
        assert N % CORES == 0
        self.N = N
        self.F_IN = F_IN
        self.F = F
        self.CORES = CORES
        self.NPC = N // CORES
        self.TILES = (self.NPC + P - 1) // P
        self.NPC_PAD = self.TILES * P
        self.TABLE_ROWS = CORES * self.NPC_PAD
        self.LOW_ROWS = min(LOW_ROWS, self.TABLE_ROWS)
        self.HIGH_ROWS = self.TABLE_ROWS - self.LOW_ROWS
        self.CHUNK_GROUPS = CHUNK_GROUPS
        self.SINGLE_PACKET = SINGLE_PACKET


def _wrap_idx_chunk(idx16):
    """int16 idx list (len % 16 == 0) -> [128, n/16] wrapped + 8x replicated."""
    n = idx16.shape[0]
    assert n % 16 == 0
    a = idx16.reshape(n // 16, 16).T  # [16, n/16]
    return np.tile(a, (8, 1)).astype(np.int16)


def _preprocess(cfg, edge_index):
    """Bucket edges by (core, src-half, dst tile); build per-core device arrays.

    Returns (meta, per_core):
      meta: program-structure constants (identical across cores)
      per_core: list of dicts of named np arrays for in_maps
    """
    src = np.asarray(edge_index[0]).astype(np.int64)
    dst = np.asarray(edge_index[1]).astype(np.int64)
    N, CORES, NPC, NPC_PAD, TILES = cfg.N, cfg.CORES, cfg.NPC, cfg.NPC_PAD, cfg.TILES

    deg = 1.0 + np.bincount(dst, minlength=N).astype(np.float32)
    dinv = (1.0 / np.sqrt(deg)).astype(np.float32)

    # Balance in-degree across the 8 cores per tile index: snake-deal nodes
    # (sorted by in-degree desc) into CORES*TILES buckets of 128 slots. This
    # equalizes per-(core,tile,stream) edge counts so the SPMD max-over-cores
    # group padding shrinks.
    NB = CORES * TILES
    order_nodes = np.argsort(-(deg), kind="stable")
    i = np.arange(N)
    rnd, idx = i // NB, i % NB
    bucket = np.where(rnd % 2 == 0, idx, NB - 1 - idx)
    slot = rnd
    assert slot.max() < P, "bucket overflow"
    c_of = bucket % CORES
    t_of = bucket // CORES
    pos = np.empty(N, np.int64)
    pos[order_nodes] = c_of * NPC_PAD + t_of * P + slot

    src_pad = pos[src]                                   # padded global row
    core_of = pos[dst] // NPC_PAD
    dst_local = pos[dst] % NPC_PAD
    tl = dst_local // P
    lane = (dst_local % P).astype(np.float32)
    hi = (src_pad >= cfg.LOW_ROWS).astype(np.int64)

    # group counts per (core, tile, stream); pad to the max across cores
    counts = np.zeros((CORES, TILES, 2), dtype=np.int64)
    np.add.at(counts, (core_of, tl, hi), 1)
    groups = np.ceil(counts / P).astype(np.int64).max(axis=0)  # [TILES, 2]
    groups[:, 0] = np.maximum(groups[:, 0], 1)  # >=1 low group per tile
    GL, GH = groups[:, 0], groups[:, 1]
    GLsum, GHsum = int(GL.sum()), int(GH.sum())
    Lg0 = np.concatenate([[0], np.cumsum(GL)])[:TILES]   # group offset per tile
    Hg0 = np.concatenate([[0], np.cumsum(GH)])[:TILES]

    # sort edges by (core, stream, tile) for contiguous slicing
    order = np.lexsort((tl, hi, core_of))
    s_src, s_hi, s_core, s_tl, s_lane = (
        src_pad[order], hi[order], core_of[order], tl[order], lane[order])

    per_core = []
    seg_starts = {}
    # compute segment boundaries: key (core, hi, tile)
    keys = s_core * (2 * TILES) + s_hi * TILES + s_tl
    uniq, first = np.unique(keys, return_index=True)
    seg_len = np.diff(np.concatenate([first, [len(keys)]]))
    for k, f, ln in zip(uniq, first, seg_len):
        seg_starts[int(k)] = (int(f), int(ln))

    for c in range(CORES):
        idxs = {0: np.zeros(GLsum * P, np.int64), 1: np.zeros(GHsum * P, np.int64)}
        lanes = {0: np.full(GLsum * P, PAD_LANE, np.float32),
                 1: np.full(GHsum * P, PAD_LANE, np.float32)}
        for s, g0s, gcnt in ((0, Lg0, GL), (1, Hg0, GH)):
            for t in range(TILES):
                key = c * (2 * TILES) + s * TILES + t
                if key not in seg_starts:
                    continue
                f, ln = seg_starts[key]
                wp = int(g0s[t]) * P
                vals = s_src[f:f + ln]
                if s == 1:
                    vals = vals - cfg.LOW_ROWS
                idxs[s][wp:wp + ln] = vals
                lanes[s][wp:wp + ln] = s_lane[f:f + ln]

        d = {}
        # wrapped idx arrays, chunked
        for s, name, gsum in ((0, "idxL", GLsum), (1, "idxH", GHsum)):
            chunks = []
            for g0 in range(0, gsum, cfg.CHUNK_GROUPS):
                g1 = min(g0 + cfg.CHUNK_GROUPS, gsum)
                chunks.append(_wrap_idx_chunk(idxs[s][g0 * P:g1 * P].astype(np.int16)))
            d[name] = (np.concatenate(chunks, axis=1) if chunks
                       else np.zeros((P, 0), np.int16))
        d["lanesL"] = np.ascontiguousarray(lanes[0].reshape(GLsum, P).T)
        d["lanesH"] = (np.ascontiguousarray(lanes[1].reshape(GHsum, P).T)
                       if GHsum else np.zeros((P, 0), np.float32))
        dl = np.ones(NPC_PAD, np.float32)
        mine = pos // NPC_PAD == c
        dl[pos[mine] % NPC_PAD] = dinv[mine]
        d["dinv"] = np.ascontiguousarray(dl.reshape(TILES, P).T)
        d["_pos"] = pos
        per_core.append(d)

    # chunk tables: (stream, ci, g0, g1); first-need tile for issue ordering
    def tile_of_group(g0s, gcnt, g):
        t = int(np.searchsorted(np.cumsum(gcnt), g, side="right"))
        return t

    chunk_list = []
    chunk_of = [{}, {}]
    for s, gsum, gcnt in ((0, GLsum, GL), (1, GHsum, GH)):
        for ci, g0 in enumerate(range(0, gsum, cfg.CHUNK_GROUPS)):
            g1 = min(g0 + cfg.CHUNK_GROUPS, gsum)
            ft = tile_of_group(None, gcnt, g0)
            chunk_list.append((s, ci, g0, g1, ft))
            for g in range(g0, g1):
                chunk_of[s][g] = (ci, g - g0)
    chunk_list.sort(key=lambda e: (e[4], e[0], e[1]))

    meta = {
        "GL": GL.tolist(), "GH": GH.tolist(),
        "Lg0": Lg0.tolist(), "Hg0": Hg0.tolist(),
        "GLsum": GLsum, "GHsum": GHsum,
        "chunks": chunk_list, "chunk_of": chunk_of, "pos": pos,
    }
    return meta, per_core, dinv


def _build_program(cfg, meta, bl_value, _mode=None):
    f32 = mybir.dt.float32
    F, F_IN, TILES, CORES = cfg.F, cfg.F_IN, cfg.TILES, cfg.CORES
    GLsum, GHsum = meta["GLsum"], meta["GHsum"]
    GL, GH = meta["GL"], meta["GH"]
    Lg0, Hg0 = meta["Lg0"], meta["Hg0"]
    KCH = F_IN // P  # K chunks for layer 1

    nc = bacc.Bacc("TRN2", target_bir_lowering=False, debug=False,
                   num_devices=CORES, num_swdge_queues=4)
    xT_d = nc.dram_tensor("xT", [F_IN, cfg.NPC_PAD], f32, kind="ExternalInput")
    w1_d = nc.dram_tensor("W1", [F_IN, F], f32, kind="ExternalInput")
    w2_d = nc.dram_tensor("W2", [F, F], f32, kind="ExternalInput")
    wl_d = nc.dram_tensor("Wl", [F, 1], f32, kind="ExternalInput")
    b1_d = nc.dram_tensor("b1t", [1, F], f32, kind="ExternalInput")
    b2_d = nc.dram_tensor("b2t", [1, F], f32, kind="ExternalInput")
    rdinv_d = nc.dram_tensor("rdinv", [1, cfg.NPC_PAD], f32, kind="ExternalInput")
    dinv_d = nc.dram_tensor("dinv", [P, TILES], f32, kind="ExternalInput")
    iota_d = nc.dram_tensor("iota", [P, 8 * P], f32, kind="ExternalInput")
    idxL_d = nc.dram_tensor("idxL", [P, GLsum * 8], mybir.dt.int16, kind="ExternalInput")
    idxH_d = nc.dram_tensor("idxH", [P, max(GHsum, 1) * 8], mybir.dt.int16, kind="ExternalInput")
    lanesL_d = nc.dram_tensor("lanesL", [P, GLsum], f32, kind="ExternalInput")
    lanesH_d = nc.dram_tensor("lanesH", [P, max(GHsum, 1)], f32, kind="ExternalInput")
    y_d = nc.dram_tensor("y", [P, TILES], f32, kind="ExternalOutput")

    with tile.TileContext(nc) as tc:
        with tc.tile_pool(name="dram", bufs=1, space="DRAM") as dpool, \
             tc.tile_pool(name="const", bufs=1) as cpool, \
             tc.tile_pool(name="hsp", bufs=TILES) as hs_pool, \
             tc.tile_pool(name="rTp", bufs=TILES) as rT_pool, \
             tc.tile_pool(name="lhs", bufs=4) as lhs_pool, \
             tc.tile_pool(name="gatL", bufs=2) as gatL_pool, \
             tc.tile_pool(name="gatH", bufs=2) as gatH_pool, \
             tc.tile_pool(name="Sp", bufs=3) as S_pool, \
             tc.tile_pool(name="ep", bufs=3) as ep_pool, \
             tc.tile_pool(name="pA", bufs=2, space="PSUM") as pA, \
             tc.tile_pool(name="pB", bufs=2, space="PSUM") as pB, \
             tc.tile_pool(name="pT", bufs=2, space="PSUM") as pT, \
             tc.tile_pool(name="pY", bufs=2, space="PSUM") as pY:

            # ---- constants ----
            w1_t = cpool.tile([P, KCH, F], f32)
            for k in range(KCH):
                nc.sync.dma_start(out=w1_t[:, k, :], in_=w1_d[k * P:(k + 1) * P, :])
            w2_t = cpool.tile([P, F], f32)
            nc.sync.dma_start(out=w2_t[:], in_=w2_d[:, :])
            wl_t = cpool.tile([P, 1], f32)
            nc.sync.dma_start(out=wl_t[:], in_=wl_d[:, :])
            b1_t = cpool.tile([1, F], f32)
            nc.sync.dma_start(out=b1_t[:], in_=b1_d[:, :])
            b2_t = cpool.tile([1, F], f32)
            nc.sync.dma_start(out=b2_t[:], in_=b2_d[:, :])
            rdinv_t = cpool.tile([1, cfg.NPC_PAD], f32)
            nc.sync.dma_start(out=rdinv_t[:], in_=rdinv_d[:, :])
            dinv_t = cpool.tile([P, TILES], f32)
            nc.sync.dma_start(out=dinv_t[:], in_=dinv_d[:, :])
            iota_t = cpool.tile([P, 8 * P], f32)
            nc.sync.dma_start(out=iota_t[:], in_=iota_d[:, :])
            idxL_t = cpool.tile([P, GLsum * 8], mybir.dt.int16)
            nc.sync.dma_start(out=idxL_t[:], in_=idxL_d[:, :])
            idxH_t = cpool.tile([P, max(GHsum, 1) * 8], mybir.dt.int16)
            nc.sync.dma_start(out=idxH_t[:], in_=idxH_d[:, :])
            lanesL_t = cpool.tile([P, GLsum], f32)
            nc.sync.dma_start(out=lanesL_t[:], in_=lanesL_d[:, :])
            lanesH_t = cpool.tile([P, max(GHsum, 1)], f32)
            nc.sync.dma_start(out=lanesH_t[:], in_=lanesH_d[:, :])
            ident = cpool.tile([P, P], f32)
            make_identity(nc, ident[:])
            ident_bf = cpool.tile([P, P], mybir.dt.bfloat16)
            make_identity(nc, ident_bf[:])

            b_tiles = [b1_t, b2_t]
            rT_tiles = [None] * TILES
            layer_bufs = []

            for l in range(2):
                ag_in = dpool.tile([cfg.NPC_PAD, F], mybir.dt.bfloat16,
                                   name=f"ag_in{l}")
                ag_out = dpool.tile([cfg.TABLE_ROWS, F], mybir.dt.bfloat16,
                                    addr_space="Shared", name=f"ag_out{l}")
                layer_bufs.append((ag_in, ag_out))

                # ---- phase A: h_s = (x @ W) * dinv, write shard table ----
                hs_tiles = []
                for t in range(TILES):
                    hpsum = pA.tile([P, F], f32, tag="hpsum", name=f"hps{l}_{t}")
                    if l == 0:
                        for k in range(KCH):
                            xt = lhs_pool.tile([P, P], f32, tag="xt",
                                               name=f"xt{l}_{t}_{k}")
                            nc.sync.dma_start(
                                out=xt[:],
                                in_=xT_d[k * P:(k + 1) * P, t * P:(t + 1) * P])
                            nc.tensor.matmul(hpsum[:], lhsT=xt[:],
                                             rhs=w1_t[:, k, :],
                                             start=(k == 0), stop=(k == KCH - 1))
                    else:
                        nc.tensor.matmul(hpsum[:], lhsT=rT_tiles[t][:],
                                         rhs=w2_t[:], start=True, stop=True)
                    hs_t = hs_pool.tile([P, F], mybir.dt.bfloat16, tag="hs",
                                        name=f"hs{l}_{t}")
                    nc.scalar.activation(
                        out=hs_t[:], in_=hpsum[:],
                        func=mybir.ActivationFunctionType.Copy,
                        scale=dinv_t[:, t:t + 1])
                    nc.sync.dma_start(out=ag_in[t * P:(t + 1) * P, :], in_=hs_t[:])
                    hs_tiles.append(hs_t)

                nc.gpsimd.collective_compute(
                    "AllGather", mybir.AluOpType.bypass,
                    replica_groups=[list(range(CORES))],
                    ins=[ag_in[:].opt()], outs=[ag_out[:].opt()])

                if _mode == "ag_only":
                    jt = cpool.tile([P, TILES], f32, name="jt")
                    nc.sync.dma_start(out=jt[:], in_=ag_out[0:P, 0:TILES])
                    nc.sync.dma_start(out=y_d[:, :], in_=jt[:])
                    break

                # ---- phase B: gather + one-hot scatter matmuls ----
                gtiles = [{}, {}]
                for qi, (s, ci, g0, g1, _ft) in enumerate(meta["chunks"]):
                    ng = g1 - g0
                    pool = gatL_pool if s == 0 else gatH_pool
                    gt = pool.tile([P, ng, F], mybir.dt.bfloat16, tag=f"g{s}",
                                   name=f"g{l}_{s}_{ci}",
                                   padded_shape=[P, cfg.CHUNK_GROUPS, F])
                    idx_t = idxL_t if s == 0 else idxH_t
                    view = (ag_out[0:cfg.LOW_ROWS, :] if s == 0
                            else ag_out[cfg.LOW_ROWS:cfg.TABLE_ROWS, :])
                    nc.gpsimd.dma_gather(
                        out_ap=gt[:], in_ap=view,
                        idxs_ap=idx_t[:, g0 * 8:g1 * 8],
                        num_idxs=ng * P, num_idxs_reg=ng * P, elem_size=F,
                        single_packet=cfg.SINGLE_PACKET,
                        queue_num=qi % 4)
                    gtiles[s][ci] = gt

                if _mode == "nomm":
                    junk = cpool.tile([P, cfg.F], f32, name="junk")
                    for s in (0, 1):
                        for gt in gtiles[s].values():
                            nc.vector.tensor_copy(out=junk[:], in_=gt[:, 0, :])
                    jt2 = cpool.tile([P, TILES], f32, name="jt2")
                    nc.vector.tensor_copy(out=jt2[:], in_=junk[:, 0:TILES])
                    nc.sync.dma_start(out=y_d[:, :], in_=jt2[:])
                    break

                Sblocks = [{}, {}]

                def get_S(s, b, l=l, Sblocks=Sblocks):
                    if b not in Sblocks[s]:
                        lan = lanesL_t if s == 0 else lanesH_t
                        gsum = GLsum if s == 0 else GHsum
                        g0, g1 = b * 8, min(b * 8 + 8, gsum)
                        st = S_pool.tile([P, (g1 - g0) * P], mybir.dt.bfloat16,
                                         tag="S", name=f"S{l}_{s}_{b}",
                                         padded_shape=[P, 8 * P])
                        nc.vector.tensor_tensor(
                            out=st[:],
                            in0=lan[:, g0:g1].to_broadcast([P, g1 - g0, P]),
                            in1=iota_t[:, :(g1 - g0) * P],
                            op=mybir.AluOpType.is_equal)
                        Sblocks[s][b] = st
                    return Sblocks[s][b]

                new_rT = [None] * TILES
                for t in range(TILES):
                    apsum = pB.tile([P, F], f32, tag="apsum", name=f"aps{l}_{t}")
                    seq = ([(0, g) for g in range(Lg0[t], Lg0[t] + GL[t])]
                           + [(1, g) for g in range(Hg0[t], Hg0[t] + GH[t])])
                    for k, (s, g) in enumerate(seq):
                        ci, slot = meta["chunk_of"][s][g]
                        st = get_S(s, g // 8)
                        j = g - (g // 8) * 8
                        nc.tensor.matmul(
                            apsum[:], lhsT=st[:, j * P:(j + 1) * P],
                            rhs=gtiles[s][ci][:, slot, :],
                            start=(k == 0), stop=False)
                    # self-loop term: psum += I @ hs  (PE accumulate)
                    nc.tensor.matmul(apsum[:], lhsT=ident_bf[:],
                                     rhs=hs_tiles[t][:], start=False, stop=False)
                    # bias pre-divided by dinv: psum += outer(1/dinv_t, b)
                    nc.tensor.matmul(apsum[:],
                                     lhsT=rdinv_t[0:1, t * P:(t + 1) * P],
                                     rhs=b_tiles[l][:], start=False, stop=True)
                    # r = relu(psum * dinv)
                    r = ep_pool.tile([P, F], f32, tag="ep4", name=f"r{l}_{t}")
                    nc.scalar.activation(out=r[:], in_=apsum[:],
                                         func=mybir.ActivationFunctionType.Relu,
                                         scale=dinv_t[:, t:t + 1])
                    tp = pT.tile([P, P], f32, tag="tp", name=f"tp{l}_{t}")
                    nc.tensor.transpose(out=tp[:], in_=r[:], identity=ident[:])
                    rT_t = rT_pool.tile([P, P], f32, tag="rT", name=f"rT{l}_{t}")
                    nc.vector.tensor_copy(out=rT_t[:], in_=tp[:])
                    new_rT[t] = rT_t
                rT_tiles = new_rT

            # ---- final linear: y = r2 @ Wl + bl ----
            if _mode is not None:
                rT_tiles = []
            y_sb = cpool.tile([P, TILES], f32, name="y_sb")
            for t in range(TILES if _mode is None else 0):
                yp = pY.tile([P, 1], f32, tag="yp", name=f"yp{t}")
                nc.tensor.matmul(yp[:], lhsT=rT_tiles[t][:], rhs=wl_t[:],
                                 start=True, stop=True)
                nc.vector.tensor_scalar(out=y_sb[:, t:t + 1], in0=yp[:],
                                        scalar1=float(bl_value), scalar2=None,
                                        op0=mybir.AluOpType.add)
            if _mode is None:
                nc.sync.dma_start(out=y_d[:, :], in_=y_sb[:])

    nc.compile()
    return nc


def _make_in_maps(cfg, per_core, x, W1, b1, W2, b2, Wl):
    iota = np.tile(np.arange(P, dtype=np.float32), (P, 8))
    b1t = np.asarray(b1, np.float32).reshape(1, -1)
    b2t = np.asarray(b2, np.float32).reshape(1, -1)
    in_maps = []
    for c in range(cfg.CORES):
        d = per_core[c]
        xT = np.zeros((cfg.F_IN, cfg.NPC_PAD), np.float32)
        pos = per_core[c]["_pos"]
        mine = pos // cfg.NPC_PAD == c
        xT[:, pos[mine] % cfg.NPC_PAD] = np.asarray(x, np.float32)[mine].T
        rdinv = (1.0 / d["dinv"]).T.reshape(1, cfg.NPC_PAD).astype(np.float32)
        gh = d["idxH"].shape[1] // 8
        in_maps.append({
            "rdinv": np.ascontiguousarray(rdinv),
            "xT": np.ascontiguousarray(xT),
            "W1": np.asarray(W1, np.float32),
            "W2": np.asarray(W2, np.float32),
            "Wl": np.asarray(Wl, np.float32).reshape(cfg.F, 1),
            "b1t": b1t, "b2t": b2t,
            "dinv": d["dinv"],
            "iota": np.ascontiguousarray(iota),
            "idxL": d["idxL"],
            "idxH": (d["idxH"] if gh else np.zeros((P, 8), np.int16)),
            "lanesL": d["lanesL"],
            "lanesH": (d["lanesH"] if d["lanesH"].shape[1]
                       else np.full((P, 1), PAD_LANE, np.float32)),
        })
    return in_maps


_CACHE = {}


def _get_compiled(cfg, edge_index):
    key = hash(np.asarray(edge_index).tobytes())
    if key not in _CACHE:
        meta, per_core, dinv = _preprocess(cfg, edge_index)
        _CACHE[key] = (meta, per_core, dinv)
    return _CACHE[key]


def kernel(x, edge_index, W1, b1, W2, b2, Wl, bl, _cfg=None, _run=None):
    cfg = _cfg or Cfg()
    x = np.asarray(x, np.float32)
    meta, per_core, _dinv = _get_compiled(cfg, edge_index)
    bl_value = float(np.asarray(bl).reshape(-1)[0])
    nc = _build_program(cfg, meta, bl_value)
    in_maps = _make_in_maps(cfg, per_core, x, W1, b1, W2, b2, Wl)
    if _run is not None:
        results = _run(nc, in_maps)
    else:
        res = bass_utils.run_bass_kernel_spmd(
            nc, in_maps, core_ids=list(range(cfg.CORES)))
        results = res.results
    pos = meta["pos"]
    y_pad = np.zeros(cfg.CORES * cfg.NPC_PAD, np.float32)
    for c in range(cfg.CORES):
        yc = results[c]["y"]  # [P, TILES]
        y_pad[c * cfg.NPC_PAD:(c + 1) * cfg.NPC_PAD] = yc.T.reshape(cfg.NPC_PAD)
    return y_pad[pos].reshape(cfg.N, 1).astype(np.float32)


# revision 17
# speedup vs baseline: 1.5786x; 1.1558x over previous
"""2-layer GCN (GCNConv -> ReLU -> GCNConv -> ReLU -> Linear) on 8 TRN2 NeuronCores.

Strategy: shard nodes (and their incident in-edges) across the 8 cores.
Per GCN layer each core computes its shard of the dinv-scaled features
h_s = (x @ W) * dinv, the shards are AllGathered into a full feature table
in DRAM, and each core aggregates messages for its destination shard by
(a) dma_gather of h_s[src] rows into SBUF (128 edges per partition-group),
(b) a one-hot scatter-matmul per 128-edge group accumulating into PSUM.
Edge lists are bucketed host-side by (dst tile, src table-half) and padded
so all 8 cores run an identical SPMD program.
"""

import numpy as np

import concourse.bass as bass
import concourse.tile as tile
from concourse import bacc, mybir, bass_utils
from concourse.masks import make_identity

P = 128
PAD_LANE = 1000.0


class Cfg:
    def __init__(self, N=50000, F_IN=256, F=128, CORES=8, LOW_ROWS=32768,
                 CHUNK_GROUPS=32, SINGLE_PACKET=False):
        assert N % CORES == 0
        self.N = N
        self.F_IN = F_IN
        self.F = F
        self.CORES = CORES
        self.NPC = N // CORES
        self.TILES = (self.NPC + P - 1) // P
        self.NPC_PAD = self.TILES * P
        self.TABLE_ROWS = CORES * self.NPC_PAD
        self.LOW_ROWS = min(LOW_ROWS, self.TABLE_ROWS)
        self.HIGH_ROWS = self.TABLE_ROWS - self.LOW_ROWS
        self.CHUNK_GROUPS = CHUNK_GROUPS
        self.SINGLE_PACKET = SINGLE_PACKET


def _wrap_idx_chunk(idx16):
    """int16 idx list (len % 16 == 0) -> [128, n/16] wrapped + 8x replicated."""
    n = idx16.shape[0]
    assert n % 16 == 0
    a = idx16.reshape(n // 16, 16).T  # [16, n/16]
    return np.tile(a, (8, 1)).astype(np.int16)


def _preprocess(cfg, edge_index):
    """Bucket edges by (core, src-half, dst tile); build per-core device arrays.

    Returns (meta, per_core):
      meta: program-structure constants (identical across cores)
      per_core: list of dicts of named np arrays for in_maps
    """
    src = np.asarray(edge_index[0]).astype(np.int64)
    dst = np.asarray(edge_index[1]).astype(np.int64)
    N, CORES, NPC, NPC_PAD, TILES = cfg.N, cfg.CORES, cfg.NPC, cfg.NPC_PAD, cfg.TILES

    deg = 1.0 + np.bincount(dst, minlength=N).astype(np.float32)
    dinv = (1.0 / np.sqrt(deg)).astype(np.float32)

    # Balance in-degree across the 8 cores per tile index: snake-deal nodes
    # (sorted by in-degree desc) into CORES*TILES buckets of 128 slots. This
    # equalizes per-(core,tile,stream) edge counts so the SPMD max-over-cores
    # group padding shrinks.
    NB = CORES * TILES
    order_nodes = np.argsort(-(deg), kind="stable")
    i = np.arange(N)
    rnd, idx = i // NB, i % NB
    bucket = np.where(rnd % 2 == 0, idx, NB - 1 - idx)
    slot = rnd
    assert slot.max() < P, "bucket overflow"
    c_of = bucket % CORES
    t_of = bucket // CORES
    pos = np.empty(N, np.int64)
    pos[order_nodes] = c_of * NPC_PAD + t_of * P + slot

    src_pad = pos[src]                                   # padded global row
    core_of = pos[dst] // NPC_PAD
    dst_local = pos[dst] % NPC_PAD
    tl = dst_local // P
    lane = (dst_local % P).astype(np.float32)
    hi = (src_pad >= cfg.LOW_ROWS).astype(np.int64)

    # group counts per (core, tile, stream); pad to the max across cores
    counts = np.zeros((CORES, TILES, 2), dtype=np.int64)
    np.add.at(counts, (core_of, tl, hi), 1)
    groups = np.ceil(counts / P).astype(np.int64).max(axis=0)  # [TILES, 2]
    groups[:, 0] = np.maximum(groups[:, 0], 1)  # >=1 low group per tile
    GL, GH = groups[:, 0], groups[:, 1]
    GLsum, GHsum = int(GL.sum()), int(GH.sum())
    Lg0 = np.concatenate([[0], np.cumsum(GL)])[:TILES]   # group offset per tile
    Hg0 = np.concatenate([[0], np.cumsum(GH)])[:TILES]

    # sort edges by (core, stream, tile) for contiguous slicing
    order = np.lexsort((tl, hi, core_of))
    s_src, s_hi, s_core, s_tl, s_lane = (
        src_pad[order], hi[order], core_of[order], tl[order], lane[order])

    per_core = []
    seg_starts = {}
    # compute segment boundaries: key (core, hi, tile)
    keys = s_core * (2 * TILES) + s_hi * TILES + s_tl
    uniq, first = np.unique(keys, return_index=True)
    seg_len = np.diff(np.concatenate([first, [len(keys)]]))
    for k, f, ln in zip(uniq, first, seg_len):
        seg_starts[int(k)] = (int(f), int(ln))

    for c in range(CORES):
        idxs = {0: np.zeros(GLsum * P, np.int64), 1: np.zeros(GHsum * P, np.int64)}
        lanes = {0: np.full(GLsum * P, PAD_LANE, np.float32),
                 1: np.full(GHsum * P, PAD_LANE, np.float32)}
        for s, g0s, gcnt in ((0, Lg0, GL), (1, Hg0, GH)):
            for t in range(TILES):
                key = c * (2 * TILES) + s * TILES + t
                if key not in seg_starts:
                    continue
                f, ln = seg_starts[key]
                wp = int(g0s[t]) * P
                vals = s_src[f:f + ln]
                if s == 1:
                    vals = vals - cfg.LOW_ROWS
                idxs[s][wp:wp + ln] = vals
                lanes[s][wp:wp + ln] = s_lane[f:f + ln]

        d = {}
        # wrapped idx arrays, chunked
        for s, name, gsum in ((0, "idxL", GLsum), (1, "idxH", GHsum)):
            chunks = []
            for g0 in range(0, gsum, cfg.CHUNK_GROUPS):
                g1 = min(g0 + cfg.CHUNK_GROUPS, gsum)
                chunks.append(_wrap_idx_chunk(idxs[s][g0 * P:g1 * P].astype(np.int16)))
            d[name] = (np.concatenate(chunks, axis=1) if chunks
                       else np.zeros((P, 0), np.int16))
        d["lanesL"] = np.ascontiguousarray(lanes[0].reshape(GLsum, P).T)
        d["lanesH"] = (np.ascontiguousarray(lanes[1].reshape(GHsum, P).T)
                       if GHsum else np.zeros((P, 0), np.float32))
        dl = np.ones(NPC_PAD, np.float32)
        mine = pos // NPC_PAD == c
        dl[pos[mine] % NPC_PAD] = dinv[mine]
        d["dinv"] = np.ascontiguousarray(dl.reshape(TILES, P).T)
        d["_pos"] = pos
        per_core.append(d)

    # chunk tables: (stream, ci, g0, g1); first-need tile for issue ordering
    def tile_of_group(g0s, gcnt, g):
        t = int(np.searchsorted(np.cumsum(gcnt), g, side="right"))
        return t

    chunk_list = []
    chunk_of = [{}, {}]
    for s, gsum, gcnt in ((0, GLsum, GL), (1, GHsum, GH)):
        for ci, g0 in enumerate(range(0, gsum, cfg.CHUNK_GROUPS)):
            g1 = min(g0 + cfg.CHUNK_GROUPS, gsum)
            ft = tile_of_group(None, gcnt, g0)
            chunk_list.append((s, ci, g0, g1, ft))
            for g in range(g0, g1):
                chunk_of[s][g] = (ci, g - g0)
    chunk_list.sort(key=lambda e: (e[4], e[0], e[1]))

    meta = {
        "GL": GL.tolist(), "GH": GH.tolist(),
        "Lg0": Lg0.tolist(), "Hg0": Hg0.tolist(),
        "GLsum": GLsum, "GHsum": GHsum,
        "chunks": chunk_list, "chunk_of": chunk_of, "pos": pos,
    }
    return meta, per_core, dinv


def _build_program(cfg, meta, bl_value, _mode=None):
    f32 = mybir.dt.float32
    F, F_IN, TILES, CORES = cfg.F, cfg.F_IN, cfg.TILES, cfg.CORES
    GLsum, GHsum = meta["GLsum"], meta["GHsum"]
    GL, GH = meta["GL"], meta["GH"]
    Lg0, Hg0 = meta["Lg0"], meta["Hg0"]
    KCH = F_IN // P  # K chunks for layer 1

    nc = bacc.Bacc("TRN2", target_bir_lowering=False, debug=False,
                   num_devices=CORES, num_swdge_queues=4)
    xT_d = nc.dram_tensor("xT", [P, KCH * TILES * P], f32, kind="ExternalInput")
    w1_d = nc.dram_tensor("W1", [F_IN, F], f32, kind="ExternalInput")
    w2_d = nc.dram_tensor("W2", [F, F], f32, kind="ExternalInput")
    wl_d = nc.dram_tensor("Wl", [F, 1], f32, kind="ExternalInput")
    b1_d = nc.dram_tensor("b1t", [1, F], f32, kind="ExternalInput")
    b2_d = nc.dram_tensor("b2t", [1, F], f32, kind="ExternalInput")
    rdinv_d = nc.dram_tensor("rdinv", [1, cfg.NPC_PAD], f32, kind="ExternalInput")
    dinv_d = nc.dram_tensor("dinv", [P, TILES], f32, kind="ExternalInput")
    iota_d = nc.dram_tensor("iota", [P, 8 * P], f32, kind="ExternalInput")
    idxL_d = nc.dram_tensor("idxL", [P, GLsum * 8], mybir.dt.int16, kind="ExternalInput")
    idxH_d = nc.dram_tensor("idxH", [P, max(GHsum, 1) * 8], mybir.dt.int16, kind="ExternalInput")
    lanesL_d = nc.dram_tensor("lanesL", [P, GLsum], f32, kind="ExternalInput")
    lanesH_d = nc.dram_tensor("lanesH", [P, max(GHsum, 1)], f32, kind="ExternalInput")
    y_d = nc.dram_tensor("y", [P, TILES], f32, kind="ExternalOutput")

    with tile.TileContext(nc) as tc:
        with tc.tile_pool(name="dram", bufs=1, space="DRAM") as dpool, \
             tc.tile_pool(name="const", bufs=1) as cpool, \
             tc.tile_pool(name="hsp", bufs=TILES) as hs_pool, \
             tc.tile_pool(name="rTp", bufs=TILES) as rT_pool, \
             tc.tile_pool(name="gatL", bufs=3) as gatL_pool, \
             tc.tile_pool(name="gatH", bufs=3) as gatH_pool, \
             tc.tile_pool(name="Sp", bufs=3) as S_pool, \
             tc.tile_pool(name="ep", bufs=3) as ep_pool, \
             tc.tile_pool(name="pA", bufs=2, space="PSUM") as pA, \
             tc.tile_pool(name="pB", bufs=2, space="PSUM") as pB, \
             tc.tile_pool(name="pT", bufs=2, space="PSUM") as pT, \
             tc.tile_pool(name="pY", bufs=2, space="PSUM") as pY:

            # ---- constants ----
            w1_t = cpool.tile([P, KCH, F], f32)
            for k in range(KCH):
                nc.sync.dma_start(out=w1_t[:, k, :], in_=w1_d[k * P:(k + 1) * P, :])
            w2_t = cpool.tile([P, F], f32)
            nc.sync.dma_start(out=w2_t[:], in_=w2_d[:, :])
            wl_t = cpool.tile([P, 1], f32)
            nc.sync.dma_start(out=wl_t[:], in_=wl_d[:, :])
            b1_t = cpool.tile([1, F], f32)
            nc.sync.dma_start(out=b1_t[:], in_=b1_d[:, :])
            b2_t = cpool.tile([1, F], f32)
            nc.sync.dma_start(out=b2_t[:], in_=b2_d[:, :])
            rdinv_t = cpool.tile([1, cfg.NPC_PAD], f32)
            nc.sync.dma_start(out=rdinv_t[:], in_=rdinv_d[:, :])
            dinv_t = cpool.tile([P, TILES], f32)
            nc.sync.dma_start(out=dinv_t[:], in_=dinv_d[:, :])
            iota_t = cpool.tile([P, 8 * P], f32)
            nc.sync.dma_start(out=iota_t[:], in_=iota_d[:, :])
            idxL_t = cpool.tile([P, GLsum * 8], mybir.dt.int16)
            nc.sync.dma_start(out=idxL_t[:], in_=idxL_d[:, :])
            idxH_t = cpool.tile([P, max(GHsum, 1) * 8], mybir.dt.int16)
            nc.sync.dma_start(out=idxH_t[:], in_=idxH_d[:, :])
            lanesL_t = cpool.tile([P, GLsum], f32)
            nc.sync.dma_start(out=lanesL_t[:], in_=lanesL_d[:, :])
            lanesH_t = cpool.tile([P, max(GHsum, 1)], f32)
            nc.sync.dma_start(out=lanesH_t[:], in_=lanesH_d[:, :])
            xTbig = cpool.tile([P, KCH * TILES * P], f32)
            nc.sync.dma_start(out=xTbig[:], in_=xT_d[:, :])
            ident = cpool.tile([P, P], f32)
            make_identity(nc, ident[:])
            ident_bf = cpool.tile([P, P], mybir.dt.bfloat16)
            make_identity(nc, ident_bf[:])

            b_tiles = [b1_t, b2_t]
            rT_tiles = [None] * TILES
            layer_bufs = []

            for l in range(2):
                ag_in = dpool.tile([cfg.NPC_PAD, F], mybir.dt.bfloat16,
                                   name=f"ag_in{l}")
                ag_out = dpool.tile([cfg.TABLE_ROWS, F], mybir.dt.bfloat16,
                                    addr_space="Shared", name=f"ag_out{l}")
                layer_bufs.append((ag_in, ag_out))

                # ---- phase A: h_s = (x @ W) * dinv, write shard table ----
                hs_tiles = []
                for t in range(TILES):
                    hpsum = pA.tile([P, F], f32, tag="hpsum", name=f"hps{l}_{t}")
                    if l == 0:
                        for k in range(KCH):
                            c0 = (k * TILES + t) * P
                            nc.tensor.matmul(hpsum[:],
                                             lhsT=xTbig[:, c0:c0 + P],
                                             rhs=w1_t[:, k, :],
                                             start=(k == 0), stop=(k == KCH - 1))
                    else:
                        nc.tensor.matmul(hpsum[:], lhsT=rT_tiles[t][:],
                                         rhs=w2_t[:], start=True, stop=True)
                    hs_t = hs_pool.tile([P, F], mybir.dt.bfloat16, tag="hs",
                                        name=f"hs{l}_{t}")
                    nc.scalar.activation(
                        out=hs_t[:], in_=hpsum[:],
                        func=mybir.ActivationFunctionType.Copy,
                        scale=dinv_t[:, t:t + 1])
                    nc.sync.dma_start(out=ag_in[t * P:(t + 1) * P, :], in_=hs_t[:])
                    hs_tiles.append(hs_t)

                nc.gpsimd.collective_compute(
                    "AllGather", mybir.AluOpType.bypass,
                    replica_groups=[list(range(CORES))],
                    ins=[ag_in[:].opt()], outs=[ag_out[:].opt()])

                if _mode == "ag_only":
                    jt = cpool.tile([P, TILES], f32, name="jt")
                    nc.sync.dma_start(out=jt[:], in_=ag_out[0:P, 0:TILES])
                    nc.sync.dma_start(out=y_d[:, :], in_=jt[:])
                    break

                # ---- phase B: gather + one-hot scatter matmuls ----
                gtiles = [{}, {}]
                for qi, (s, ci, g0, g1, _ft) in enumerate(meta["chunks"]):
                    ng = g1 - g0
                    pool = gatL_pool if s == 0 else gatH_pool
                    gt = pool.tile([P, ng, F], mybir.dt.bfloat16, tag=f"g{s}",
                                   name=f"g{l}_{s}_{ci}",
                                   padded_shape=[P, cfg.CHUNK_GROUPS, F])
                    idx_t = idxL_t if s == 0 else idxH_t
                    view = (ag_out[0:cfg.LOW_ROWS, :] if s == 0
                            else ag_out[cfg.LOW_ROWS:cfg.TABLE_ROWS, :])
                    nc.gpsimd.dma_gather(
                        out_ap=gt[:], in_ap=view,
                        idxs_ap=idx_t[:, g0 * 8:g1 * 8],
                        num_idxs=ng * P, num_idxs_reg=ng * P, elem_size=F,
                        single_packet=cfg.SINGLE_PACKET,
                        queue_num=qi % 4)
                    gtiles[s][ci] = gt

                if _mode == "nomm":
                    junk = cpool.tile([P, cfg.F], f32, name="junk")
                    for s in (0, 1):
                        for gt in gtiles[s].values():
                            nc.vector.tensor_copy(out=junk[:], in_=gt[:, 0, :])
                    jt2 = cpool.tile([P, TILES], f32, name="jt2")
                    nc.vector.tensor_copy(out=jt2[:], in_=junk[:, 0:TILES])
                    nc.sync.dma_start(out=y_d[:, :], in_=jt2[:])
                    break

                Sblocks = [{}, {}]

                def get_S(s, b, l=l, Sblocks=Sblocks):
                    if b not in Sblocks[s]:
                        lan = lanesL_t if s == 0 else lanesH_t
                        gsum = GLsum if s == 0 else GHsum
                        g0, g1 = b * 8, min(b * 8 + 8, gsum)
                        st = S_pool.tile([P, (g1 - g0) * P], mybir.dt.bfloat16,
                                         tag="S", name=f"S{l}_{s}_{b}",
                                         padded_shape=[P, 8 * P])
                        nc.vector.tensor_tensor(
                            out=st[:],
                            in0=lan[:, g0:g1].to_broadcast([P, g1 - g0, P]),
                            in1=iota_t[:, :(g1 - g0) * P],
                            op=mybir.AluOpType.is_equal)
                        Sblocks[s][b] = st
                    return Sblocks[s][b]

                new_rT = [None] * TILES
                for t in range(TILES):
                    apsum = pB.tile([P, F], f32, tag="apsum", name=f"aps{l}_{t}")
                    seq = ([(0, g) for g in range(Lg0[t], Lg0[t] + GL[t])]
                           + [(1, g) for g in range(Hg0[t], Hg0[t] + GH[t])])
                    for k, (s, g) in enumerate(seq):
                        ci, slot = meta["chunk_of"][s][g]
                        st = get_S(s, g // 8)
                        j = g - (g // 8) * 8
                        nc.tensor.matmul(
                            apsum[:], lhsT=st[:, j * P:(j + 1) * P],
                            rhs=gtiles[s][ci][:, slot, :],
                            start=(k == 0), stop=False)
                    # self-loop term: psum += I @ hs  (PE accumulate)
                    nc.tensor.matmul(apsum[:], lhsT=ident_bf[:],
                                     rhs=hs_tiles[t][:], start=False, stop=False)
                    # bias pre-divided by dinv: psum += outer(1/dinv_t, b)
                    nc.tensor.matmul(apsum[:],
                                     lhsT=rdinv_t[0:1, t * P:(t + 1) * P],
                                     rhs=b_tiles[l][:], start=False, stop=True)
                    # r = relu(psum * dinv)
                    r = ep_pool.tile([P, F], f32, tag="ep4", name=f"r{l}_{t}")
                    nc.scalar.activation(out=r[:], in_=apsum[:],
                                         func=mybir.ActivationFunctionType.Relu,
                                         scale=dinv_t[:, t:t + 1])
                    tp = pT.tile([P, P], f32, tag="tp", name=f"tp{l}_{t}")
                    nc.tensor.transpose(out=tp[:], in_=r[:], identity=ident[:])
                    rT_t = rT_pool.tile([P, P], f32, tag="rT", name=f"rT{l}_{t}")
                    nc.vector.tensor_copy(out=rT_t[:], in_=tp[:])
                    new_rT[t] = rT_t
                rT_tiles = new_rT

            # ---- final linear: y = r2 @ Wl + bl ----
            if _mode is not None:
                rT_tiles = []
            y_sb = cpool.tile([P, TILES], f32, name="y_sb")
            for t in range(TILES if _mode is None else 0):
                yp = pY.tile([P, 1], f32, tag="yp", name=f"yp{t}")
                nc.tensor.matmul(yp[:], lhsT=rT_tiles[t][:], rhs=wl_t[:],
                                 start=True, stop=True)
                nc.vector.tensor_scalar(out=y_sb[:, t:t + 1], in0=yp[:],
                                        scalar1=float(bl_value), scalar2=None,
                                        op0=mybir.AluOpType.add)
            if _mode is None:
                nc.sync.dma_start(out=y_d[:, :], in_=y_sb[:])

    nc.compile()
    return nc


def _make_in_maps(cfg, per_core, x, W1, b1, W2, b2, Wl):
    iota = np.tile(np.arange(P, dtype=np.float32), (P, 8))
    b1t = np.asarray(b1, np.float32).reshape(1, -1)
    b2t = np.asarray(b2, np.float32).reshape(1, -1)
    in_maps = []
    for c in range(cfg.CORES):
        d = per_core[c]
        pos = per_core[c]["_pos"]
        mine = pos // cfg.NPC_PAD == c
        x_pad = np.zeros((cfg.NPC_PAD, cfg.F_IN), np.float32)
        x_pad[pos[mine] % cfg.NPC_PAD] = np.asarray(x, np.float32)[mine]
        KCH = cfg.F_IN // P
        arr = x_pad.reshape(cfg.TILES, P, KCH, P)
        xT = np.ascontiguousarray(
            arr.transpose(3, 2, 0, 1).reshape(P, KCH * cfg.TILES * P))
        rdinv = (1.0 / d["dinv"]).T.reshape(1, cfg.NPC_PAD).astype(np.float32)
        gh = d["idxH"].shape[1] // 8
        in_maps.append({
            "rdinv": np.ascontiguousarray(rdinv),
            "xT": np.ascontiguousarray(xT),
            "W1": np.asarray(W1, np.float32),
            "W2": np.asarray(W2, np.float32),
            "Wl": np.asarray(Wl, np.float32).reshape(cfg.F, 1),
            "b1t": b1t, "b2t": b2t,
            "dinv": d["dinv"],
            "iota": np.ascontiguousarray(iota),
            "idxL": d["idxL"],
            "idxH": (d["idxH"] if gh else np.zeros((P, 8), np.int16)),
            "lanesL": d["lanesL"],
            "lanesH": (d["lanesH"] if d["lanesH"].shape[1]
                       else np.full((P, 1), PAD_LANE, np.float32)),
        })
    return in_maps


_CACHE = {}


def _get_compiled(cfg, edge_index):
    key = hash(np.asarray(edge_index).tobytes())
    if key not in _CACHE:
        meta, per_core, dinv = _preprocess(cfg, edge_index)
        _CACHE[key] = (meta, per_core, dinv)
    return _CACHE[key]


def kernel(x, edge_index, W1, b1, W2, b2, Wl, bl, _cfg=None, _run=None):
    cfg = _cfg or Cfg()
    x = np.asarray(x, np.float32)
    meta, per_core, _dinv = _get_compiled(cfg, edge_index)
    bl_value = float(np.asarray(bl).reshape(-1)[0])
    nc = _build_program(cfg, meta, bl_value)
    in_maps = _make_in_maps(cfg, per_core, x, W1, b1, W2, b2, Wl)
    if _run is not None:
        results = _run(nc, in_maps)
    else:
        res = bass_utils.run_bass_kernel_spmd(
            nc, in_maps, core_ids=list(range(cfg.CORES)))
        results = res.results
    pos = meta["pos"]
    y_pad = np.zeros(cfg.CORES * cfg.NPC_PAD, np.float32)
    for c in range(cfg.CORES):
        yc = results[c]["y"]  # [P, TILES]
        y_pad[c * cfg.NPC_PAD:(c + 1) * cfg.NPC_PAD] = yc.T.reshape(cfg.NPC_PAD)
    return y_pad[pos].reshape(cfg.N, 1).astype(np.float32)
